# revision 62
# baseline (speedup 1.0000x reference)
"""Trainium2 Bass kernel for nn_CNN_52604759441677 (dense_cnn).

Model (eval forward):
  emb_out   = word_emb[words]                                   [S,B,300]
  char_feat = max_t(groupedConv1d(char_emb[chars]) + b)         [B,S,150]
  cnn_input = concat(emb_out, char_feat)                        [B,S,450]
  base      = cnn_input @ Ww^T                                  [B,S,64]
  pos_part  = pos_emb[|s-i|] @ Wp^T                             [S,S,64]
  y[i,b]    = base[b] + pos_part[i] + lin_b                     [S,64] image
  feats     = for j in 3: max_h relu(conv2d(y, K_j 64x64) + cb) [S*B,192]
  out       = feats @ fc_w^T + fc_b                             [S,B,20]

Key algebraic structure exploited:
 1. conv2d is linear: conv(y[i,b]) = conv(base_b) + conv(pos_i + lin_b).
 2. pos images are shifts of ONE weight-only sequence qext[d] = q[|d|], so
    conv(pos_i)[h] = C0[h-i] -- C0 is a small input-independent constant
    computed on host (pure weight folding, lin_b included).
 3. word_emb only feeds base through Ww, so wv = word_emb @ Ww^T is
    pre-folded; the device gathers its 96 rows in one indirect DMA.
 4. char conv taps are stacked on the contraction axis (est tensor), making
    the grouped conv a single matmul per t with contraction 90, in
    word-major orientation so max-over-t reduces use all lanes.

Sharding: data-parallel over the batch axis B: core b handles image b for
ALL 96 anchor positions (the anchor side of the conv is the host-folded
constant C0, so replicating it is free).  Everything per-word (char conv,
base, embedding gather, est upload) shards 8x with the batch.  The only
non-sharded work is the (i,h) combine, which is invariant to how the 768
(i,b) pairs are split across cores.  No collectives.
"""

import sys

sys.path.insert(0, "/opt/trn_rl_repo")

import numpy as np

from concourse import bass, mybir
from concourse import bass_utils
from concourse.ap import AP
from concourse.vector_clock import ScopedClock
import concourse.tile as tile

F16 = mybir.dt.float16
F32 = mybir.dt.float32
I32 = mybir.dt.int32

# Model dims
S, B, W = 96, 8, 15
VOCAB, EMB = 50000, 300
CHAR_VOCAB, CH_EMB = 100, 30
FILT, CH_K = 5, 3
CH_OUT = CH_EMB * FILT  # 150
POS_N, POS_D = 128, 25
C = 64
N_K = 3
OUT = 20
H = S - C + 1  # 33 conv output positions

N_CORES = 8
NWC = S  # words per core (one batch image)
T13 = W - CH_K + 1  # 13 char-conv positions
KC = CH_K * CH_EMB  # 90 char-conv contraction
ND = 128  # C0 columns: d = h - i in [-95, 32]

# blob16 column layout
B16_IDENT = 0
B16_WBLK = 128
B16_WCA = B16_WBLK + CH_OUT      # 278
B16_WCB = B16_WCA + C            # 342
B16_C0A = B16_WCB + C            # 406
B16_C0B = B16_C0A + ND           # 534
B16_FCA = B16_C0B + ND           # 662
B16_FCB = B16_FCA + OUT          # 682
B16_FCBIAS = B16_FCB + OUT       # 702
B16_ONES = B16_FCBIAS + OUT      # 722
NB16 = B16_ONES + S              # 818

N_WARM = 6  # PE p-state warm-up matmuls


def _patch_tile_drain():
    """Walrus in this container rejects >1 sem wait on InstDrain ("Too many
    sync wait commands"). Emit the waits as individual SP wait_ge
    instructions before an unadorned drain instead."""
    if getattr(tile.TileContext, "_drain_patched", False):
        return

    def _patched(self, tick_clock, wait_clock):
        nc = self.nc
        probe = nc.sync.nop()
        wait_clock.add_sem_waits(
            probe.ins, ScopedClock({None: tick_clock.global_clock})
        )
        si = probe.ins.sync_info
        waits = list(si.on_wait) if si is not None and si.on_wait else []
        if si is not None:
            si.on_wait = []
        num_to_handle = {h.num: h for h in self.sems.allocated().values()}
        for wv in waits:
            assert wv.wait_mode == "sem-ge-imm", wv
            h = num_to_handle.get(wv.id)
            assert h is not None, f"no sem handle for wait {wv}"
            nc.sync.wait_ge(h, wv.wait_value)
        nc.sync.drain()
        nc.all_engine_barrier()
        assert self.sems is not None
        popped = nc._tile_sem_poison_stack.pop()
        assert popped is self._sem_poison
        nc.clear_and_free_semaphores(list(self.sems.allocated().values()))

    tile.TileContext._drain_and_barrier = _patched
    tile.TileContext._drain_patched = True


def _split_excess_waits(nc, max_waits=1):
    """TRN2/walrus rejects >2 sem waits on one instruction. Move excess
    waits onto InstEventSemaphore instructions inserted just before."""
    n_split = 0
    for f in nc.m.functions:
        for blk in f.blocks:
            insts = list(blk.instructions)
            out = []
            for inst in insts:
                si = inst.sync_info
                waits = list(si.on_wait) if si is not None and si.on_wait else []
                if len(waits) > max_waits:
                    extra = waits[: len(waits) - 1]
                    keep = waits[len(waits) - 1:]
                    si.on_wait = keep
                    for j in range(0, len(extra), 1):
                        evs = mybir.InstNoOp(
                            name=f"evsplit-{nc.next_id()}", ins=[], outs=[]
                        )
                        evs.engine = inst.engine
                        evs.sync_info = mybir.SyncInfo(
                            on_wait=extra[j:j + 1], on_update=[]
                        )
                        out.append(evs)
                        n_split += 1
                out.append(inst)
            if n_split:
                blk.instructions = out
    return n_split


def build_program(split_waits=True, debug=False):
    _patch_tile_drain()
    nc = bass.Bass()

    def inp(name, shape, dt):
        return nc.declare_dram_parameter(name, list(shape), dt, isOutput=False)

    wv = inp("wv", [VOCAB, C], F16)           # word_emb @ Ww^T (DRAM, gathered)
    est = inp("est", [KC, NWC * T13], F16)    # this core's char embeddings
    cvw01 = inp("cvw01", [128, 32 * 128], F16)
    cvw2 = inp("cvw2", [128, 32 * 128], F16)  # w2 duplicated (lane packing)
    blob16 = inp("blob16", [128, NB16], F16)
    blob32 = inp("blob32", [128, 8], F32)

    out_d = nc.declare_dram_parameter("out", [S, OUT], F32, isOutput=True)

    with tile.TileContext(nc) as tc:
        with (
            tc.tile_pool(name="persist", bufs=1) as pp,
            tc.tile_pool(name="scratch", bufs=2) as sp,
            tc.tile_pool(name="psB", bufs=1, space="PSUM") as ps_b,
        ):
            # ---- parameter loads -----------------------------------------
            # SP: est, w01a, w2 ; Act: blob16, blob32, w01b
            blob16_sb = pp.tile([128, NB16], F16, tag="b16", name="b16")
            nc.scalar.dma_start(blob16_sb[:], blob16[:])
            blob32_sb = pp.tile([128, 8], F32, tag="b32", name="b32")
            nc.scalar.dma_start(blob32_sb[:], blob32[:])
            # Pre-load the Act "Copy" table while DMAs stream so the first
            # real copy doesn't pay ACT_TABLE_LOAD on the spine.
            # (separate tile: the warm-up matmuls read wsc)
            dmy = pp.tile([1, 2], F16, tag="dmy", name="dmy")
            nc.vector.memset(dmy[:], 0)
            nc.scalar.copy(dmy[0:1, 0:1], dmy[0:1, 1:2])
            wsc = pp.tile([128, 128], F16, tag="wsc", name="wsc")
            nc.vector.memset(wsc[:], 0)
            est_sb = pp.tile([KC, NWC * T13], F16, tag="est", name="est")
            nc.sync.dma_start(est_sb[:, 0:7 * NWC], est[:, 0:7 * NWC])
            nc.sync.dma_start(est_sb[:, 7 * NWC:], est[:, 7 * NWC:])
            cvw01_sb = pp.tile([128, 32 * 128], F16, tag="w01", name="w01")
            nc.sync.dma_start(cvw01_sb[:, 0:2048], cvw01[:, 0:2048])
            nc.scalar.dma_start(cvw01_sb[:, 2048:4096], cvw01[:, 2048:4096])
            cvw2_sb = pp.tile([128, 32 * 128], F16, tag="w2", name="w2")
            nc.sync.dma_start(cvw2_sb[:], cvw2[:])

            ident_sb = blob16_sb[0:NWC, B16_IDENT:B16_IDENT + NWC]
            wblk_sb = blob16_sb[0:KC, B16_WBLK:B16_WBLK + CH_OUT]
            wca_sb = blob16_sb[0:128, B16_WCA:B16_WCA + C]
            wcb_sb = blob16_sb[0:CH_OUT - 128, B16_WCB:B16_WCB + C]

            # ---- word-side gather (wv rows for this image) ---------------
            wvsb = pp.tile([NWC, C], F16, tag="wvsb", name="wvsb")
            nc.gpsimd.indirect_dma_start(
                out=wvsb[:],
                out_offset=None,
                in_=wv[:],
                in_offset=bass.IndirectOffsetOnAxis(
                    ap=blob32_sb[0:NWC, 2:3].bitcast(I32), axis=0
                ),
            )

            charT_a = pp.tile([128, NWC], F16, tag="chTa", name="chTa")
            charT_b = pp.tile([CH_OUT - 128, NWC], F16, tag="chTb", name="chTb")
            xtB = pp.tile([128, NWC], F16, tag="xtB", name="xtB")

            # ---- char conv + max-over-t (one 96-word chunk) --------------
            estv = est_sb[:].rearrange("p (t n) -> p t n", t=T13)
            with (
                tc.tile_pool(name="ct", bufs=1, space="PSUM") as ps_ct,
                tc.tile_pool(name="wm", bufs=1, space="PSUM") as ps_w,
            ):
                # PE p-state warm-up: dummy matmuls on the zeroed tile
                for wi in range(N_WARM):
                    wps = ps_w.tile([128, 128], F32, tag="wm", name="warm",
                                    space="PSUM")
                    nc.tensor.matmul(wps[:], wsc[:], wsc[:],
                                     start=True, stop=True)

                # single 5-bank tile: t0-11 in banks 0-3, t12 + transpose
                # scratch in bank 4
                TAll = ps_ct.tile([128, 3072], F32, tag="ct", name="TAll",
                                  space="PSUM")
                TC = TAll[0:NWC, 2048:2048 + CH_OUT]

                def tslice(t):
                    if t < 12:
                        return TAll[0:NWC,
                                    (t // 3) * 512 + (t % 3) * CH_OUT:
                                    (t // 3) * 512 + (t % 3) * CH_OUT + CH_OUT]
                    return TC

                for t in range(T13):
                    nc.tensor.matmul(tslice(t), estv[:, t, :], wblk_sb,
                                     start=True, stop=True)
                # one XY-reduce over all 12 bank-packed taps, then t12
                chf = pp.tile([NWC, CH_OUT], F16, tag="chf", name="chf")
                tap = TAll[0:NWC, :]
                v4 = AP(tap.tensor, tap.offset,
                        [[tap.ap[0][0], NWC], [1, CH_OUT],
                         [512, 4], [CH_OUT, 3]])
                nc.vector.tensor_reduce(out=chf[:], in_=v4,
                                        axis=mybir.AxisListType.XY,
                                        op=mybir.AluOpType.max)
                nc.vector.tensor_tensor(out=chf[:], in0=chf[:], in1=TC,
                                        op=mybir.AluOpType.max)

                # transposes back to [gf, n] (one psum group in bank 4)
                tpa = TAll[:, 2200:2264].bitcast(F16)[0:128, 0:NWC]
                tpb = TAll[:, 2560:2624].bitcast(F16)[0:CH_OUT - 128, 0:NWC]
                nc.tensor.matmul(tpa, chf[:, 0:128], ident_sb,
                                 is_transpose=True, start=True, stop=True)
                nc.tensor.matmul(tpb, chf[:, 128:CH_OUT], ident_sb,
                                 is_transpose=True, start=True, stop=True)
                nc.scalar.copy(charT_a[:, 0:NWC], tpa)
                nc.scalar.copy(charT_b[:, 0:NWC], tpb)

                # ---- base (wv transpose + char matmuls in one psum) ------
                bp = ps_b.tile([C, NWC], F32, tag="psB", name="bp",
                               space="PSUM")
                nc.tensor.matmul(bp[:], wvsb[:], ident_sb,
                                 start=True, stop=False)
                nc.tensor.matmul(bp[:], wca_sb, charT_a[:, 0:NWC],
                                 start=False, stop=False)
                nc.tensor.matmul(bp[:], wcb_sb, charT_b[:, 0:NWC],
                                 start=False, stop=True)
                nc.scalar.copy(xtB[0:C, 0:NWC], bp[:])
                # rows 64..127 = base shifted by one position (s+1)
                nc.scalar.copy(xtB[C:128, 0:NWC - 1], bp[:, 1:NWC])

            # ---- B-side conv (this image only) + combine -----------------
            w01v = cvw01_sb[:].rearrange("r (p m) -> r p m", p=32)
            w2v = cvw2_sb[:].rearrange("r (p m) -> r p m", p=32)
            with (
                tc.tile_pool(name="cv01", bufs=2, space="PSUM") as ps_cv1,
                tc.tile_pool(name="cv2", bufs=1, space="PSUM") as ps_cv2,
                tc.tile_pool(name="po", bufs=1, space="PSUM") as ps_o,
            ):
                # group2 is lane-packed: rows 64:127 handle i' 64:96 via
                # a host-shifted C0 copy (64 pair-columns on 128 lanes).
                # Both groups' V ranges live in ONE tile so a single
                # halving tree reduces all 160 pair-columns at once.
                NPAIR = S + C  # 96 (group01) + 64 (group2 packed)
                vsc = sp.tile([128, NPAIR * H], F16, tag="vsc", name="vsc")
                pmq = sp.tile([128, NPAIR], F16, tag="pm", name="pm")
                for (gi, wvw, npairs, poff, c0col, hpool) in (
                    (0, w01v, S, 0, B16_C0A, ps_cv1),
                    (1, w2v, C, S, B16_C0B, ps_cv2),
                ):
                    # group01's conv runs as two h-half chains (separate
                    # banks) so its first combine add starts while the
                    # second half-chain is still accumulating.
                    hsplit = ((0, 17), (17, 16)) if gi == 0 else ((0, H),)
                    cvb = sp.tile([128, H], F16, tag=f"cvb{gi}",
                                  name=f"cvb{gi}")
                    c0ap = blob16_sb[0:128, c0col:c0col + ND]
                    cvap = cvb[:]
                    off = poff * H
                    for (h0, hl) in hsplit:
                        cp = hpool.tile([128, H], F32, tag=f"cv{gi}",
                                        name=f"cv{gi}", space="PSUM")
                        for p in range(32):
                            nc.tensor.matmul(
                                cp[:, 0:hl], wvw[:, p, :],
                                xtB[:, 2 * p + h0:2 * p + h0 + hl],
                                start=(p == 0), stop=(p == 31))
                        nc.scalar.copy(cvb[:, h0:h0 + hl], cp[:, 0:hl])
                        # V[i',h] = C0[i'+h] + convB[h] for this h-range
                        vv = AP(vsc[:].tensor, vsc[:].offset + off + h0,
                                [[vsc[:].ap[0][0], 128], [H, npairs],
                                 [1, hl]])
                        d = AP(c0ap.tensor, c0ap.offset + h0,
                               [[c0ap.ap[0][0], 128], [1, npairs], [1, hl]])
                        bb = AP(cvap.tensor, cvap.offset + h0,
                                [[cvap.ap[0][0], 128], [0, npairs], [1, hl]])
                        nc.vector.tensor_tensor(out=vv, in0=d, in1=bb,
                                                op=mybir.AluOpType.add)
                # in-place TT-max halving tree over h (16,8,4,2,1) for all
                # 160 pair-columns of both groups at once
                v3 = vsc[:].rearrange("p (n h) -> p n h", h=H)
                w = 16
                nc.vector.tensor_tensor(
                    out=v3[:, :, 0:w], in0=v3[:, :, 0:w],
                    in1=v3[:, :, w:2 * w], op=mybir.AluOpType.max)
                while w > 1:
                    w //= 2
                    nc.vector.tensor_tensor(
                        out=v3[:, :, 0:w], in0=v3[:, :, 0:w],
                        in1=v3[:, :, w:2 * w], op=mybir.AluOpType.max)
                nc.vector.tensor_tensor(
                    out=pmq[:], in0=v3[:, :, 0].squeeze(),
                    in1=v3[:, :, 32].squeeze(), op=mybir.AluOpType.max)
                # relu(+conv bias) without touching Act tables
                fT = []
                for gi, poff, npairs, cbias in (
                    (0, 0, S, blob32_sb[0:128, 0:1]),
                    (1, S, C, blob32_sb[0:128, 1:2]),
                ):
                    ft = pp.tile([128, S], F16, tag=f"fT{gi}",
                                 name=f"fT{gi}")
                    nc.vector.tensor_scalar(
                        out=ft[:, 0:npairs], in0=pmq[:, poff:poff + npairs],
                        scalar1=cbias, scalar2=0.0,
                        op0=mybir.AluOpType.add, op1=mybir.AluOpType.max)
                    if gi == 1:
                        # unpack lanes 64:127 (pairs 64:96) back to rows 0:64
                        nc.vector.tensor_copy(ft[0:C, C:S],
                                              ft[C:128, 0:S - C])
                    fT.append(ft)

                # ---- fc -------------------------------------------------
                po = ps_o.tile([S, OUT], F32, tag="po", name="po",
                               space="PSUM")
                nc.tensor.matmul(po[:], fT[0][:],
                                 blob16_sb[0:128, B16_FCA:B16_FCA + OUT],
                                 start=True, stop=False)
                nc.tensor.matmul(po[:], fT[1][0:C, :],
                                 blob16_sb[0:C, B16_FCB:B16_FCB + OUT],
                                 start=False, stop=False)
                nc.tensor.matmul(po[:],
                                 blob16_sb[0:1, B16_ONES:B16_ONES + S],
                                 blob16_sb[0:1, B16_FCBIAS:B16_FCBIAS + OUT],
                                 start=False, stop=True)
                out_sb = pp.tile([S, OUT], F32, tag="outsb", name="outsb")
                nc.vector.tensor_copy(out_sb[:], po[:])
                nc.sync.dma_start(out_d[:], out_sb[:])

    if split_waits:
        _split_excess_waits(nc)
    return nc


def host_prep(inputs):
    """Build shared + per-core input maps from the full model inputs."""
    words = np.asarray(inputs["words"]).astype(np.int64)  # [S,B]
    chars = np.asarray(inputs["chars"]).astype(np.int64)  # [B,S,W]
    word_emb = np.asarray(inputs["word_emb"], np.float64)
    char_emb = np.asarray(inputs["char_emb"], np.float32)
    char_cnn_w = np.asarray(inputs["char_cnn_w"], np.float64)[:, 0, :]  # [150,3]
    char_cnn_b = np.asarray(inputs["char_cnn_b"], np.float64)
    pos_emb = np.asarray(inputs["pos_emb"], np.float64)
    lin_w = np.asarray(inputs["lin_w"], np.float64)
    lin_b = np.asarray(inputs["lin_b"], np.float64)
    conv_w = np.asarray(inputs["conv_w"], np.float64)  # [3,64,1,64,64]
    conv_b = np.asarray(inputs["conv_b"], np.float32)  # [3,64]
    fc_w = np.asarray(inputs["fc_w"], np.float32)  # [20,192]
    fc_b = np.asarray(inputs["fc_b"], np.float32)

    We = lin_w[:, :EMB]              # [64, 300]
    Wc = lin_w[:, EMB:EMB + CH_OUT]  # [64, 150]
    Wp = lin_w[:, EMB + CH_OUT:]     # [64, 25]

    shared = {}
    # ---- word side: fold We into the embedding table ---------------------
    shared["wv"] = (word_emb @ We.T).astype(np.float16)  # [50000, 64]

    # ---- char side: tap-stacked char embeddings + blockdiag weights ------
    chars_n = chars.reshape(B * S, W)  # row n = b*96+s
    e = char_emb[chars_n]  # [768, 15, 30] f32
    est = np.empty((CH_K, CH_EMB, B * S, T13), np.float16)
    for k in range(CH_K):
        est[k] = e[:, k:k + T13, :].transpose(2, 0, 1)
    est_full = est.reshape(KC, B, S, T13)
    wblk = np.zeros((CH_K, CH_EMB, CH_OUT), np.float64)
    for k in range(CH_K):
        for gf in range(CH_OUT):
            wblk[k, gf // FILT, gf] = char_cnn_w[gf, k]
    wblk150 = wblk.reshape(KC, CH_OUT).astype(np.float16)

    # ---- pos side: fold conv over qext into the constant C0 --------------
    beff = lin_b + Wc @ char_cnn_b  # char bias folded (max_t(cc)+b)
    q = pos_emb[:S] @ Wp.T          # [96, 64]
    dd = np.arange(-95, 96)
    yP = q[np.abs(dd)] + beff       # [191, 64]
    K3 = conv_w[:, :, 0, :, :]      # [3, co, dh, c]
    d_vals = np.arange(-95, 33)
    idx = d_vals[:, None] + np.arange(C)[None, :] + 95  # [128, 64]
    Yw = yP[idx]                    # [128(d), 64(dh), 64(c)]
    C0 = np.einsum('jodc,ndc->jon', K3, Yw)  # [3, 64, 128(d)]
    # V[i',h] uses col i'+h with i = 95-i'  (d = h-i = i'+h-95)

    # ---- conv weights (2-tap row-dup packing), flat layout ---------------
    w01 = np.zeros((32, 128, 128), np.float16)
    w2 = np.zeros((32, 128, C), np.float16)
    cwf = conv_w.astype(np.float16)
    for p in range(32):
        for ei in range(2):
            blk = cwf[:, :, 0, 2 * p + ei, :]  # [j, co, dw]
            w01[p, ei * C:(ei + 1) * C, 0:C] = blk[0].T
            w01[p, ei * C:(ei + 1) * C, C:128] = blk[1].T
            w2[p, ei * C:(ei + 1) * C, :] = blk[2].T
    shared["cvw01"] = w01.transpose(1, 0, 2).reshape(128, 32 * 128).copy()
    w2d = np.concatenate([w2, w2], axis=2)  # [32, 128, 128] lane-duplicated
    shared["cvw2"] = w2d.transpose(1, 0, 2).reshape(128, 32 * 128).copy()

    # ---- blob16 (shared) -------------------------------------------------
    blob16 = np.zeros((128, NB16), np.float16)
    blob16[:, B16_IDENT:B16_IDENT + 128] = np.eye(128, dtype=np.float16)
    blob16[0:KC, B16_WBLK:B16_WBLK + CH_OUT] = wblk150
    blob16[0:128, B16_WCA:B16_WCA + C] = Wc[:, 0:128].T.astype(np.float16)
    blob16[0:CH_OUT - 128, B16_WCB:B16_WCB + C] = \
        Wc[:, 128:CH_OUT].T.astype(np.float16)
    blob16[0:128, B16_C0A:B16_C0A + ND] = np.concatenate(
        [C0[0], C0[1]], axis=0).astype(np.float16)
    # group2 C0, lane-packed: rows 64:127 are shifted by 64 anchor rows
    c0b2 = np.zeros((128, 96), np.float64)
    c0b2[0:C, :] = C0[2][:, 0:96]
    c0b2[C:128, 0:C] = C0[2][:, C:ND]
    blob16[0:128, B16_C0B:B16_C0B + 96] = c0b2.astype(np.float16)
    blob16[0:128, B16_FCA:B16_FCA + OUT] = fc_w[:, 0:128].T.astype(np.float16)
    blob16[0:C, B16_FCB:B16_FCB + OUT] = fc_w[:, 128:192].T.astype(np.float16)
    blob16[C:128, B16_FCB:B16_FCB + OUT] = \
        fc_w[:, 128:192].T.astype(np.float16)
    blob16[0:1, B16_FCBIAS:B16_FCBIAS + OUT] = fc_b.reshape(1, OUT)
    blob16[0:1, B16_ONES:B16_ONES + S] = 1.0

    in_maps = []
    for core in range(N_CORES):
        m = dict(shared)
        m["blob16"] = blob16
        m["est"] = np.ascontiguousarray(
            est_full[:, core].transpose(0, 2, 1).reshape(KC, T13 * S))
        blob32 = np.zeros((128, 8), np.float32)
        blob32[:, 0] = conv_b[0:2].reshape(128)
        blob32[0:C, 1] = conv_b[2]
        blob32[C:128, 1] = conv_b[2]
        blob32[0:S, 2] = words[:, core].astype(np.int32).view(np.float32)
        m["blob32"] = blob32
        in_maps.append(m)
    return in_maps


_CACHE = {}


def kernel(**inputs) -> np.ndarray:
    if "nc" not in _CACHE:
        _CACHE["nc"] = build_program()
    nc = _CACHE["nc"]
    in_maps = host_prep(inputs)
    res = bass_utils.run_bass_kernel_spmd(
        nc, in_maps, core_ids=list(range(N_CORES))
    )
    out = np.zeros((S, B, OUT), np.float32)
    for core in range(N_CORES):
        blk = res.results[core]["out"]  # [96 (i' = 95-i), 20]
        out[:, core, :] = blk[::-1]
    return out
